# revision 29
# baseline (speedup 1.0000x reference)
"""Trainium2 Bass kernel for nn_ActorWrapper (GNN message-passing heads).

Reference computation (per full batch):
  stem_preds  = MLP_s(per_atom_out[slices[stems_batch] + stems])        [131072, 105]
  bonds_preds = pair-mean of MLP_b(per_atom_out[bond endpoint rows])    [516096, 1]
  per_mol_out passthrough.

Strategy (8 NeuronCores, data-parallel by molecule; core c owns rows
[c*65536, (c+1)*65536) of per_atom_out, converted to bf16 on host):

  Bonds head (the bulk): computed DENSELY over every shard row — the
  1-wide second layer means each row yields one scalar P_b[r]; the host
  then forms each bond as 0.5*(P_b[a]+P_b[b]) + b_b2 by direct indexing.
  This removes ~1M gathered-row reads (each endpoint row was fetched
  ~2x) and replaces them with one contiguous streaming pass in a
  host-pre-transposed feature-major layout [128, 2, 65536].

  Stems head: gathered on device via dma_gather(transpose=True), which
  lands rows directly in K-on-partitions layout. dma_gather indices are
  int16, so rows are bucketed into two 32768-row windows per core; each
  gather call handles one 512-token tile (the SDMA descriptor ring
  limits a transpose gather to ~1008 indices). Stems' second layer is a
  [105, 512] matmul; bias fused via ScalarE Identity activation.

  Both heads: bf16 matmuls, f32 PSUM accumulation, bias+LeakyReLU fused
  into one ScalarE activation per 128-feature chunk (some tiles routed
  to VectorE via a 3-op max(y, 0.01y) sequence to balance engine load).
  Bonds' 1-wide layer 2 runs as a swapped-roles matmul (h-slice
  stationary, w column moving) packing token-major [128,1] columns into
  PSUM at even fp32 offsets (8B cachelines).

  No collectives: per-core outputs are disjoint.
"""

import numpy as np
import ml_dtypes
from contextlib import ExitStack

import concourse.bass as bass
import concourse.mybir as mybir
import concourse.tile as tile
from concourse import bacc
from concourse.bass_utils import run_bass_kernel_spmd

BF16 = ml_dtypes.bfloat16

# Lrelu on HW; debug scripts may switch to Relu (CoreSim lacks Lrelu).
ACT_FUNC = mybir.ActivationFunctionType.Lrelu
IS_RELU = False  # kept in sync with ACT_FUNC by debug scripts

NCORES = 8
B = 8192
APM = 64
DIM = 256
MOLS_PER_CORE = B // NCORES          # 1024
ROWS_PER_CORE = MOLS_PER_CORE * APM  # 65536
BUCKET = 32768                       # int16 index reach per dma_gather
MM_N = 512                           # tokens per matmul tile
DCHUNK = 4096                        # dense rows per input DMA
S_DIM2 = 105
N_DTILES = ROWS_PER_CORE // MM_N     # 128 dense tiles
N_BGROUPS = N_DTILES // 64           # bond output groups (64 tiles each)
# route every DVE_EVERY-th tile's L1 activation to VectorE
DVE_EVERY = 4
# stems output staging width in 512-token tiles
SO_TILES = 8

NEG_SLOPE = 0.01


def _round_up(x, m):
    return ((x + m - 1) // m) * m


def _wrap16(a):
    """Index array [n] (n % 16 == 0) -> [128, n/16] int16 (token j at
    [j % 16, j // 16], replicated for the 8 gpsimd cores)."""
    w = a.reshape(-1, 16).T
    return np.ascontiguousarray(np.tile(w, (8, 1)).astype(np.int16))


def build_graph(ns_buckets):
    """Per-core Bacc graph. ns_buckets: per-bucket padded stem token
    counts (multiples of 512)."""
    NST = sum(ns_buckets)
    n_stiles = NST // MM_N

    nc = bacc.Bacc()
    table = nc.declare_dram_parameter("table", [ROWS_PER_CORE, DIM], mybir.dt.bfloat16, isOutput=False)
    x_t = nc.declare_dram_parameter("x_t", [128, 2, ROWS_PER_CORE], mybir.dt.bfloat16, isOutput=False)
    stem_idx = nc.declare_dram_parameter("stem_idx", [128, NST // 16], mybir.dt.int16, isOutput=False)
    w1s = nc.declare_dram_parameter("w1s", [128, 2, DIM], mybir.dt.bfloat16, isOutput=False)
    w1b = nc.declare_dram_parameter("w1b", [128, 2, DIM], mybir.dt.bfloat16, isOutput=False)
    w2s = nc.declare_dram_parameter("w2s", [128, 2, S_DIM2], mybir.dt.bfloat16, isOutput=False)
    w2b = nc.declare_dram_parameter("w2b", [128, 2, 1], mybir.dt.bfloat16, isOutput=False)
    b1s = nc.declare_dram_parameter("b1s", [128, 2], mybir.dt.float32, isOutput=False)
    b1b = nc.declare_dram_parameter("b1b", [128, 2], mybir.dt.float32, isOutput=False)
    bs2 = nc.declare_dram_parameter("bs2", [S_DIM2, 1], mybir.dt.float32, isOutput=False)
    stem_out = nc.declare_dram_parameter("stem_out", [S_DIM2, NST], mybir.dt.float32, isOutput=True)
    bond_out = nc.declare_dram_parameter("bond_out", [N_BGROUPS, 128, 256], mybir.dt.float32, isOutput=True)

    with tile.TileContext(nc) as tc, ExitStack() as ctx:
        const_p = ctx.enter_context(tc.tile_pool(name="const", bufs=1))
        xd_p = ctx.enter_context(tc.tile_pool(name="xd", bufs=2))
        xg_p = ctx.enter_context(tc.tile_pool(name="xg", bufs=6))
        h_p = ctx.enter_context(tc.tile_pool(name="h", bufs=4))
        tmp_p = ctx.enter_context(tc.tile_pool(name="tmp", bufs=2))
        so_p = ctx.enter_context(tc.tile_pool(name="so", bufs=2))
        bo_p = ctx.enter_context(tc.tile_pool(name="bo", bufs=2))
        ph_p = ctx.enter_context(tc.tile_pool(name="ph", bufs=3, space="PSUM"))
        ps2_p = ctx.enter_context(tc.tile_pool(name="ps2", bufs=1, space="PSUM"))
        bp_p = ctx.enter_context(tc.tile_pool(name="bp", bufs=1, space="PSUM"))

        # constants
        def const_tile(name, param, shape, dt, engine=None):
            t = const_p.tile(shape, dt, tag=name, name=name + "_t")
            (engine or nc.sync).dma_start(t[:], param[:])
            return t

        # bonds-path consts first: the Sync HWDGE FIFO serializes DMAs, and
        # the dense stream's first chunk should land as early as possible.
        w1b_t = const_tile("w1b", w1b, [128, 2, DIM], mybir.dt.bfloat16)
        b1b_t = const_tile("b1b", b1b, [128, 2], mybir.dt.float32)
        w2b_t = const_tile("w2b", w2b, [128, 2, 1], mybir.dt.bfloat16)
        deferred_consts = {}

        def late_consts():
            deferred_consts["sidx"] = const_tile("sidx", stem_idx, [128, NST // 16], mybir.dt.int16)
            deferred_consts["w1s"] = const_tile("w1s", w1s, [128, 2, DIM], mybir.dt.bfloat16)
            deferred_consts["b1s"] = const_tile("b1s", b1s, [128, 2], mybir.dt.float32)
            deferred_consts["w2s"] = const_tile("w2s", w2s, [128, 2, S_DIM2], mybir.dt.bfloat16)
            deferred_consts["bs2"] = const_tile("bs2", bs2, [S_DIM2, 1], mybir.dt.float32)

        def l1(rhs_fn, w1_t, b1_t, on_dve):
            """First layer + bias + LeakyReLU for one 512-token tile.
            rhs_fn(k) yields the [128, 512] K-chunk moving operand.
            Returns h tile [128, 2, MM_N] bf16."""
            h = h_p.tile([128, 2, MM_N], mybir.dt.bfloat16, tag="h")
            pss = [ph_p.tile([128, MM_N], mybir.dt.float32, tag=f"ph{m}", name=f"ph{m}")
                   for m in range(2)]
            for m in range(2):
                for k in range(2):
                    nc.tensor.matmul(
                        out=pss[m][:],
                        lhsT=w1_t[:, k, m * 128 : (m + 1) * 128],
                        rhs=rhs_fn(k),
                        start=(k == 0), stop=(k == 1),
                    )
            for m in range(2):
                psl = pss[m][:]
                if on_dve:
                    y = tmp_p.tile([128, MM_N], mybir.dt.float32, tag="yv", name="yv")
                    nc.vector.tensor_scalar_add(out=y[:], in0=psl, scalar1=b1_t[:, m : m + 1])
                    t = tmp_p.tile([128, MM_N], mybir.dt.bfloat16, tag="tv", name="tv")
                    nc.vector.tensor_scalar_mul(out=t[:], in0=y[:], scalar1=NEG_SLOPE)
                    nc.vector.tensor_tensor(
                        out=h[:, m, :], in0=y[:], in1=t[:], op=mybir.AluOpType.max)
                else:
                    nc.scalar.activation(
                        out=h[:, m, :], in_=psl, func=ACT_FUNC,
                        bias=b1_t[:, m : m + 1], scale=1.0, alpha=NEG_SLOPE)
            return h

        # ---------------- stems tile emission (interleaved) ----------------
        stem_work = []  # (bucket, offset_within_bucket, global_tile_idx)
        ti = 0
        for b, nbk in enumerate(ns_buckets):
            for off in range(0, nbk, MM_N):
                stem_work.append((b, off, ti))
                ti += 1
        stem_state = {"stage": None, "pend": None}

        def emit_stem_l1(work):
            b, off, tglob = work
            base = sum(ns_buckets[:b])
            src = table[b * BUCKET : (b + 1) * BUCKET, :]
            xg = xg_p.tile([128, 2, MM_N], mybir.dt.bfloat16, tag="xg")
            col0 = (base + off) // 16
            nc.gpsimd.dma_gather(
                out_ap=xg[:], in_ap=src,
                idxs_ap=deferred_consts["sidx"][:, col0 : col0 + MM_N // 16],
                num_idxs=MM_N, num_idxs_reg=MM_N,
                elem_size=DIM, transpose=True,
            )
            h = l1(lambda k: xg[:, k, :],
                   deferred_consts["w1s"], deferred_consts["b1s"],
                   on_dve=False)
            return (h, tglob)

        def emit_stem_l2(pend):
            h, tglob = pend
            ps2 = ps2_p.tile([S_DIM2, MM_N], mybir.dt.float32, tag="ps2", name="ps2")
            for k in range(2):
                nc.tensor.matmul(
                    out=ps2[:], lhsT=deferred_consts["w2s"][:, k, :], rhs=h[:, k, :],
                    start=(k == 0), stop=(k == 1),
                )
            if tglob % SO_TILES == 0:
                stem_state["stage"] = so_p.tile(
                    [S_DIM2, SO_TILES * MM_N], mybir.dt.float32, tag="so", name="so_stage")
            c0 = (tglob % SO_TILES) * MM_N
            nc.scalar.activation(
                out=stem_state["stage"][:, c0 : c0 + MM_N], in_=ps2[:],
                func=mybir.ActivationFunctionType.Identity,
                bias=deferred_consts["bs2"][:, :1], scale=1.0,
            )
            if tglob % SO_TILES == SO_TILES - 1 or tglob == n_stiles - 1:
                w = c0 + MM_N
                out0 = (tglob - tglob % SO_TILES) * MM_N
                nc.sync.dma_start(stem_out[:, out0 : out0 + w], stem_state["stage"][:, :w])

        def emit_stem_tile(work):
            # 1-stage software pipeline: L2 of the previous stems tile is
            # emitted after this tile's L1, so PE isn't blocked on the
            # activation of the tile it just multiplied.
            nxt = emit_stem_l1(work)
            if stem_state["pend"] is not None:
                emit_stem_l2(stem_state["pend"])
            stem_state["pend"] = nxt

        # ---------------- dense bonds stream with stems interleaved --------
        chunks = []
        pos = 0
        ramp = [512, 1024, 2048]
        while pos < ROWS_PER_CORE:
            sz = ramp.pop(0) if ramp else DCHUNK
            sz = min(sz, ROWS_PER_CORE - pos)
            chunks.append((pos, sz))
            pos += sz
        # Don't interleave stems into the first chunks: their gathers (and
        # sidx load) aren't done yet and would head-of-line-block the PE.
        STEM_START_CHUNK = 2
        eligible = max(1, len(chunks) - STEM_START_CHUNK)
        stems_per_chunk = _round_up(len(stem_work), eligible) // eligible
        sw = 0
        bond_state = {"bp": None, "pend": []}

        def emit_bond_l2(pend):
            h, tglob = pend
            if tglob % 64 == 0:
                bond_state["bp"] = bp_p.tile([128, MM_N], mybir.dt.float32, tag="bp", name="bp")
            bp_ps = bond_state["bp"]
            c0 = (tglob % 64) * 8
            for s in range(4):
                for k in range(2):
                    nc.tensor.matmul(
                        out=bp_ps[:, c0 + 2 * s : c0 + 2 * s + 1],
                        lhsT=h[:, k, s * 128 : (s + 1) * 128],
                        rhs=w2b_t[:, k, :],
                        start=(k == 0), stop=(k == 1),
                    )
            if tglob % 64 == 63:
                g = tglob // 64
                bo = bo_p.tile([128, 256], mybir.dt.float32, tag="bo", name="bo")
                bp_even = bp_ps[:].rearrange("p (n two) -> p n two", two=2)
                nc.vector.tensor_copy(bo[:], bp_even[:, :, 0])
                nc.sync.dma_start(bond_out[g], bo[:])

        for ci, (cstart, csz) in enumerate(chunks):
            xd = xd_p.tile([128, 2, csz], mybir.dt.bfloat16, tag="xd", name="xd")
            nc.sync.dma_start(xd[:], x_t[:, :, cstart : cstart + csz])
            if ci == 0:
                late_consts()
            for t0 in range(0, csz, MM_N):
                tglob = (cstart + t0) // MM_N
                h = l1(lambda k, _xd=xd, _t0=t0: _xd[:, k, _t0 : _t0 + MM_N],
                       w1b_t, b1b_t,
                       on_dve=(tglob % DVE_EVERY == DVE_EVERY - 1))
                bond_state["pend"].append((h, tglob))
                if len(bond_state["pend"]) > 1:
                    emit_bond_l2(bond_state["pend"].pop(0))
            if ci >= STEM_START_CHUNK:
                for _ in range(stems_per_chunk):
                    if sw < len(stem_work):
                        emit_stem_tile(stem_work[sw])
                        sw += 1
        while sw < len(stem_work):
            emit_stem_tile(stem_work[sw])
            sw += 1
        for pend in bond_state["pend"]:
            emit_bond_l2(pend)
        if stem_state["pend"] is not None:
            emit_stem_l2(stem_state["pend"])

    nc.compile()
    return nc


_GRAPH_CACHE = {}


def _get_graph(ns_buckets):
    key = tuple(ns_buckets)
    if key not in _GRAPH_CACHE:
        _GRAPH_CACHE[key] = build_graph(ns_buckets)
    return _GRAPH_CACHE[key]


def kernel(per_atom_out, per_mol_out, w_s1, b_s1, w_s2, b_s2,
           w_b1, b_b1, w_b2, b_b2, slices, stems_batch, stems,
           bonds, bonds_batch):
    per_atom_out = np.asarray(per_atom_out, dtype=np.float32)
    per_mol_out = np.asarray(per_mol_out)
    slices_np = np.asarray(slices, dtype=np.int64)
    stems_batch = np.asarray(stems_batch, dtype=np.int64)
    stems = np.asarray(stems, dtype=np.int64)
    bonds = np.asarray(bonds, dtype=np.int64)
    bonds_batch = np.asarray(bonds_batch, dtype=np.int64)
    n_stems = stems_batch.shape[0]
    n_bonds = bonds_batch.shape[0]

    # ---- host: per-core stem binning ----
    stem_gidx = slices_np[stems_batch] + stems
    owner_s = stem_gidx // ROWS_PER_CORE

    per_core = []
    for c in range(NCORES):
        sel_s = np.flatnonzero(owner_s == c)
        loc_s = stem_gidx[sel_s] - c * ROWS_PER_CORE
        bkt_s = loc_s >> 15
        o = np.argsort(bkt_s, kind="stable")
        sel_s, loc_s = sel_s[o], loc_s[o]
        ns0 = int((loc_s < BUCKET).sum())
        per_core.append((sel_s, loc_s, ns0))

    NS0 = max(512, _round_up(max(pc[2] for pc in per_core), 512))
    NS1 = max(512, _round_up(max(len(pc[1]) - pc[2] for pc in per_core), 512))
    NST = NS0 + NS1

    # ---- host: weights/bias prep (shared across cores) ----
    w_s1 = np.asarray(w_s1, np.float32); w_b1 = np.asarray(w_b1, np.float32)
    w_s2 = np.asarray(w_s2, np.float32); w_b2 = np.asarray(w_b2, np.float32)
    b_s1 = np.asarray(b_s1, np.float32); b_b1 = np.asarray(b_b1, np.float32)
    b_s2 = np.asarray(b_s2, np.float32); b_b2 = np.asarray(b_b2, np.float32)

    def prep_w(w, d2):
        return np.ascontiguousarray(
            w.T.reshape(2, 128, d2).transpose(1, 0, 2).astype(BF16))

    w1s_a = prep_w(w_s1, DIM)
    w1b_a = prep_w(w_b1, DIM)
    w2s_a = prep_w(w_s2, S_DIM2)
    w2b_a = prep_w(0.5 * w_b2, 1)
    b1s_a = np.ascontiguousarray(b_s1.reshape(2, 128).T.astype(np.float32))
    b1b_a = np.ascontiguousarray(b_b1.reshape(2, 128).T.astype(np.float32))
    bs2_a = np.ascontiguousarray(b_s2.reshape(S_DIM2, 1).astype(np.float32))

    table_bf16 = per_atom_out.astype(BF16)

    # ---- host: per-core in_maps ----
    in_maps = []
    for c in range(NCORES):
        sel_s, loc_s, ns0 = per_core[c]
        ns1 = len(loc_s) - ns0

        sidx = np.zeros(NST, np.int16)
        sidx[:ns0] = loc_s[:ns0] & (BUCKET - 1)
        sidx[NS0 : NS0 + ns1] = loc_s[ns0:] & (BUCKET - 1)

        shard = table_bf16[c * ROWS_PER_CORE : (c + 1) * ROWS_PER_CORE]
        xt = np.ascontiguousarray(
            shard.T.reshape(2, 128, ROWS_PER_CORE).transpose(1, 0, 2))

        in_maps.append({
            "table": shard,
            "x_t": xt,
            "stem_idx": _wrap16(sidx),
            "w1s": w1s_a, "w1b": w1b_a, "w2s": w2s_a, "w2b": w2b_a,
            "b1s": b1s_a, "b1b": b1b_a, "bs2": bs2_a,
        })

    nc = _get_graph((NS0, NS1))
    global LAST_RUN
    LAST_RUN = (nc, in_maps)
    results = run_bass_kernel_spmd(nc, in_maps, core_ids=list(range(NCORES))).results

    # ---- host: assemble outputs ----
    stem_preds = np.empty((n_stems, S_DIM2), np.float32)
    P = np.empty(NCORES * ROWS_PER_CORE, np.float32)
    for c in range(NCORES):
        sel_s, loc_s, ns0 = per_core[c]
        ns1 = len(loc_s) - ns0
        so = results[c]["stem_out"]          # [105, NST]
        colmap = np.concatenate([np.arange(ns0), NS0 + np.arange(ns1)])
        stem_preds[sel_s] = so[:, colmap].T

        bo = results[c]["bond_out"]          # [G, 128, 256]
        P[c * ROWS_PER_CORE : (c + 1) * ROWS_PER_CORE] = (
            bo.transpose(0, 2, 1).reshape(-1))

    ga = slices_np[bonds_batch] + bonds[:, 0]
    gb = slices_np[bonds_batch] + bonds[:, 1]
    bonds_preds = (P[ga] + P[gb] + b_b2[0]).reshape(-1, 1).astype(np.float32)

    return (stem_preds, per_mol_out, bonds_preds)


# revision 30
# speedup vs baseline: 1.1310x; 1.1310x over previous
"""Trainium2 Bass kernel for nn_ActorWrapper (GNN message-passing heads).

Reference computation (per full batch):
  stem_preds  = MLP_s(per_atom_out[slices[stems_batch] + stems])        [131072, 105]
  bonds_preds = pair-mean of MLP_b(per_atom_out[bond endpoint rows])    [516096, 1]
  per_mol_out passthrough.

Strategy (8 NeuronCores, data-parallel by molecule; core c owns rows
[c*65536, (c+1)*65536) of per_atom_out, converted to bf16 on host):

  Bonds head (the bulk): computed DENSELY over every shard row — the
  1-wide second layer means each row yields one scalar P_b[r]; the host
  then forms each bond as 0.5*(P_b[a]+P_b[b]) + b_b2 by direct indexing.
  This removes ~1M gathered-row reads (each endpoint row was fetched
  ~2x) and replaces them with one contiguous streaming pass in a
  host-pre-transposed feature-major layout [128, 2, 65536].

  Stems head: gathered on device via dma_gather(transpose=True), which
  lands rows directly in K-on-partitions layout. dma_gather indices are
  int16, so rows are bucketed into two 32768-row windows per core; each
  gather call handles one 512-token tile (the SDMA descriptor ring
  limits a transpose gather to ~1008 indices). Stems' second layer is a
  [105, 512] matmul; bias fused via ScalarE Identity activation.

  Both heads: bf16 matmuls, f32 PSUM accumulation, bias+LeakyReLU fused
  into one ScalarE activation per 128-feature chunk (some tiles routed
  to VectorE via a 3-op max(y, 0.01y) sequence to balance engine load).
  Bonds' 1-wide layer 2 runs as a swapped-roles matmul (h-slice
  stationary, w column moving) packing token-major [128,1] columns into
  PSUM at even fp32 offsets (8B cachelines).

  No collectives: per-core outputs are disjoint.
"""

import numpy as np
import ml_dtypes
from contextlib import ExitStack

import concourse.bass as bass
import concourse.mybir as mybir
import concourse.tile as tile
from concourse import bacc
from concourse.bass_utils import run_bass_kernel_spmd

BF16 = ml_dtypes.bfloat16

# Lrelu on HW; debug scripts may switch to Relu (CoreSim lacks Lrelu).
ACT_FUNC = mybir.ActivationFunctionType.Lrelu
IS_RELU = False  # kept in sync with ACT_FUNC by debug scripts

NCORES = 8
B = 8192
APM = 64
DIM = 256
MOLS_PER_CORE = B // NCORES          # 1024
ROWS_PER_CORE = MOLS_PER_CORE * APM  # 65536
BUCKET = 32768                       # int16 index reach per dma_gather
MM_N = 512                           # tokens per matmul tile
DCHUNK = 4096                        # dense rows per input DMA
S_DIM2 = 105
N_DTILES = ROWS_PER_CORE // MM_N     # 128 dense tiles
N_BGROUPS = N_DTILES // 64           # bond output groups (64 tiles each)
# route every DVE_EVERY-th tile's L1 activation to VectorE
DVE_EVERY = 4
# stems output staging width in 512-token tiles
SO_TILES = 8

NEG_SLOPE = 0.01


def _round_up(x, m):
    return ((x + m - 1) // m) * m


def _wrap16(a):
    """Index array [n] (n % 16 == 0) -> [128, n/16] int16 (token j at
    [j % 16, j // 16], replicated for the 8 gpsimd cores)."""
    w = a.reshape(-1, 16).T
    return np.ascontiguousarray(np.tile(w, (8, 1)).astype(np.int16))


def build_graph(ns_buckets):
    """Per-core Bacc graph. ns_buckets: per-bucket padded stem token
    counts (multiples of 512)."""
    NST = sum(ns_buckets)
    n_stiles = NST // MM_N

    nc = bacc.Bacc()
    table = nc.declare_dram_parameter("table", [ROWS_PER_CORE, DIM], mybir.dt.bfloat16, isOutput=False)
    x_t = nc.declare_dram_parameter("x_t", [128, 2, ROWS_PER_CORE], mybir.dt.bfloat16, isOutput=False)
    stem_idx = nc.declare_dram_parameter("stem_idx", [128, NST // 16], mybir.dt.int16, isOutput=False)
    w1s = nc.declare_dram_parameter("w1s", [128, 2, DIM], mybir.dt.bfloat16, isOutput=False)
    w1b = nc.declare_dram_parameter("w1b", [128, 2, DIM], mybir.dt.bfloat16, isOutput=False)
    w2s = nc.declare_dram_parameter("w2s", [128, 2, S_DIM2], mybir.dt.bfloat16, isOutput=False)
    w2b = nc.declare_dram_parameter("w2b", [128, 2, 1], mybir.dt.bfloat16, isOutput=False)
    b1s = nc.declare_dram_parameter("b1s", [128, 2], mybir.dt.float32, isOutput=False)
    b1b = nc.declare_dram_parameter("b1b", [128, 2], mybir.dt.float32, isOutput=False)
    bs2 = nc.declare_dram_parameter("bs2", [S_DIM2, 1], mybir.dt.float32, isOutput=False)
    stem_out = nc.declare_dram_parameter("stem_out", [S_DIM2, NST], mybir.dt.float32, isOutput=True)
    bond_out = nc.declare_dram_parameter("bond_out", [N_BGROUPS, 128, 256], mybir.dt.float32, isOutput=True)

    with tile.TileContext(nc) as tc, ExitStack() as ctx:
        const_p = ctx.enter_context(tc.tile_pool(name="const", bufs=1))
        xd_p = ctx.enter_context(tc.tile_pool(name="xd", bufs=2))
        xg_p = ctx.enter_context(tc.tile_pool(name="xg", bufs=6))
        h_p = ctx.enter_context(tc.tile_pool(name="h", bufs=4))
        tmp_p = ctx.enter_context(tc.tile_pool(name="tmp", bufs=2))
        so_p = ctx.enter_context(tc.tile_pool(name="so", bufs=2))
        bo_p = ctx.enter_context(tc.tile_pool(name="bo", bufs=2))
        ph_p = ctx.enter_context(tc.tile_pool(name="ph", bufs=3, space="PSUM"))
        ps2_p = ctx.enter_context(tc.tile_pool(name="ps2", bufs=1, space="PSUM"))
        bp_p = ctx.enter_context(tc.tile_pool(name="bp", bufs=1, space="PSUM"))

        # constants
        def const_tile(name, param, shape, dt, engine=None):
            t = const_p.tile(shape, dt, tag=name, name=name + "_t")
            (engine or nc.sync).dma_start(t[:], param[:])
            return t

        # bonds-path consts first: the Sync HWDGE FIFO serializes DMAs, and
        # the dense stream's first chunk should land as early as possible.
        w1b_t = const_tile("w1b", w1b, [128, 2, DIM], mybir.dt.bfloat16)
        b1b_t = const_tile("b1b", b1b, [128, 2], mybir.dt.float32)
        w2b_t = const_tile("w2b", w2b, [128, 2, 1], mybir.dt.bfloat16)
        deferred_consts = {}

        def late_consts():
            deferred_consts["sidx"] = const_tile("sidx", stem_idx, [128, NST // 16], mybir.dt.int16)
            deferred_consts["w1s"] = const_tile("w1s", w1s, [128, 2, DIM], mybir.dt.bfloat16)
            deferred_consts["b1s"] = const_tile("b1s", b1s, [128, 2], mybir.dt.float32)
            deferred_consts["w2s"] = const_tile("w2s", w2s, [128, 2, S_DIM2], mybir.dt.bfloat16)
            deferred_consts["bs2"] = const_tile("bs2", bs2, [S_DIM2, 1], mybir.dt.float32)

        def l1(rhs_fn, w1_t, b1_t, on_dve):
            """First layer + bias + LeakyReLU for one 512-token tile.
            rhs_fn(k) yields the [128, 512] K-chunk moving operand.
            Returns h tile [128, 2, MM_N] bf16."""
            h = h_p.tile([128, 2, MM_N], mybir.dt.bfloat16, tag="h")
            pss = [ph_p.tile([128, MM_N], mybir.dt.float32, tag=f"ph{m}", name=f"ph{m}")
                   for m in range(2)]
            for m in range(2):
                for k in range(2):
                    nc.tensor.matmul(
                        out=pss[m][:],
                        lhsT=w1_t[:, k, m * 128 : (m + 1) * 128],
                        rhs=rhs_fn(k),
                        start=(k == 0), stop=(k == 1),
                    )
            for m in range(2):
                psl = pss[m][:]
                if on_dve:
                    y = tmp_p.tile([128, MM_N], mybir.dt.float32, tag="yv", name="yv")
                    nc.vector.tensor_scalar_add(out=y[:], in0=psl, scalar1=b1_t[:, m : m + 1])
                    t = tmp_p.tile([128, MM_N], mybir.dt.bfloat16, tag="tv", name="tv")
                    nc.vector.tensor_scalar_mul(out=t[:], in0=y[:], scalar1=NEG_SLOPE)
                    nc.vector.tensor_tensor(
                        out=h[:, m, :], in0=y[:], in1=t[:], op=mybir.AluOpType.max)
                else:
                    nc.scalar.activation(
                        out=h[:, m, :], in_=psl, func=ACT_FUNC,
                        bias=b1_t[:, m : m + 1], scale=1.0, alpha=NEG_SLOPE)
            return h

        # ---------------- stems tile emission (interleaved) ----------------
        stem_work = []  # (bucket, offset_within_bucket, global_tile_idx)
        ti = 0
        for b, nbk in enumerate(ns_buckets):
            for off in range(0, nbk, MM_N):
                stem_work.append((b, off, ti))
                ti += 1
        stem_state = {"stage": None, "pend": None}

        def emit_stem_l1(work):
            b, off, tglob = work
            base = sum(ns_buckets[:b])
            src = table[b * BUCKET : (b + 1) * BUCKET, :]
            xg = xg_p.tile([128, 2, MM_N], mybir.dt.bfloat16, tag="xg")
            col0 = (base + off) // 16
            nc.gpsimd.dma_gather(
                out_ap=xg[:], in_ap=src,
                idxs_ap=deferred_consts["sidx"][:, col0 : col0 + MM_N // 16],
                num_idxs=MM_N, num_idxs_reg=MM_N,
                elem_size=DIM, transpose=True,
            )
            h = l1(lambda k: xg[:, k, :],
                   deferred_consts["w1s"], deferred_consts["b1s"],
                   on_dve=False)
            return (h, tglob)

        def emit_stem_l2(pend):
            h, tglob = pend
            ps2 = ps2_p.tile([S_DIM2, MM_N], mybir.dt.float32, tag="ps2", name="ps2")
            for k in range(2):
                nc.tensor.matmul(
                    out=ps2[:], lhsT=deferred_consts["w2s"][:, k, :], rhs=h[:, k, :],
                    start=(k == 0), stop=(k == 1),
                )
            if tglob % SO_TILES == 0:
                stem_state["stage"] = so_p.tile(
                    [S_DIM2, SO_TILES * MM_N], mybir.dt.float32, tag="so", name="so_stage")
            c0 = (tglob % SO_TILES) * MM_N
            nc.vector.tensor_scalar_add(
                out=stem_state["stage"][:, c0 : c0 + MM_N], in0=ps2[:],
                scalar1=deferred_consts["bs2"][:, :1],
            )
            if tglob % SO_TILES == SO_TILES - 1 or tglob == n_stiles - 1:
                w = c0 + MM_N
                out0 = (tglob - tglob % SO_TILES) * MM_N
                nc.sync.dma_start(stem_out[:, out0 : out0 + w], stem_state["stage"][:, :w])

        def emit_stem_tile(work):
            # 1-stage software pipeline: L2 of the previous stems tile is
            # emitted after this tile's L1, so PE isn't blocked on the
            # activation of the tile it just multiplied.
            nxt = emit_stem_l1(work)
            if stem_state["pend"] is not None:
                emit_stem_l2(stem_state["pend"])
            stem_state["pend"] = nxt

        # ---------------- dense bonds stream with stems interleaved --------
        chunks = [(0, 512)]
        pos = 512
        while pos < ROWS_PER_CORE:
            sz = min(DCHUNK, ROWS_PER_CORE - pos)
            chunks.append((pos, sz))
            pos += sz
        # Don't interleave stems into the first chunks: their gathers (and
        # sidx load) aren't done yet and would head-of-line-block the PE.
        STEM_START_CHUNK = 2
        eligible = max(1, len(chunks) - STEM_START_CHUNK)
        stems_per_chunk = _round_up(len(stem_work), eligible) // eligible
        sw = 0
        bond_state = {"bp": None, "pend": []}

        def emit_bond_l2(pend):
            h, tglob = pend
            if tglob % 64 == 0:
                bond_state["bp"] = bp_p.tile([128, MM_N], mybir.dt.float32, tag="bp", name="bp")
            bp_ps = bond_state["bp"]
            c0 = (tglob % 64) * 8
            for s in range(4):
                for k in range(2):
                    nc.tensor.matmul(
                        out=bp_ps[:, c0 + 2 * s : c0 + 2 * s + 1],
                        lhsT=h[:, k, s * 128 : (s + 1) * 128],
                        rhs=w2b_t[:, k, :],
                        start=(k == 0), stop=(k == 1),
                    )
            if tglob % 64 == 63:
                g = tglob // 64
                bo = bo_p.tile([128, 256], mybir.dt.float32, tag="bo", name="bo")
                bp_even = bp_ps[:].rearrange("p (n two) -> p n two", two=2)
                nc.vector.tensor_copy(bo[:], bp_even[:, :, 0])
                nc.sync.dma_start(bond_out[g], bo[:])

        for ci, (cstart, csz) in enumerate(chunks):
            xd = xd_p.tile([128, 2, csz], mybir.dt.bfloat16, tag="xd", name="xd")
            nc.sync.dma_start(xd[:], x_t[:, :, cstart : cstart + csz])
            if ci == 0:
                late_consts()
            for t0 in range(0, csz, MM_N):
                tglob = (cstart + t0) // MM_N
                h = l1(lambda k, _xd=xd, _t0=t0: _xd[:, k, _t0 : _t0 + MM_N],
                       w1b_t, b1b_t,
                       on_dve=(tglob % DVE_EVERY == DVE_EVERY - 1))
                bond_state["pend"].append((h, tglob))
                if len(bond_state["pend"]) > 1:
                    emit_bond_l2(bond_state["pend"].pop(0))
            if ci >= STEM_START_CHUNK:
                for _ in range(stems_per_chunk):
                    if sw < len(stem_work):
                        emit_stem_tile(stem_work[sw])
                        sw += 1
        while sw < len(stem_work):
            emit_stem_tile(stem_work[sw])
            sw += 1
        for pend in bond_state["pend"]:
            emit_bond_l2(pend)
        if stem_state["pend"] is not None:
            emit_stem_l2(stem_state["pend"])

    nc.compile()
    return nc


_GRAPH_CACHE = {}


def _get_graph(ns_buckets):
    key = tuple(ns_buckets)
    if key not in _GRAPH_CACHE:
        _GRAPH_CACHE[key] = build_graph(ns_buckets)
    return _GRAPH_CACHE[key]


def kernel(per_atom_out, per_mol_out, w_s1, b_s1, w_s2, b_s2,
           w_b1, b_b1, w_b2, b_b2, slices, stems_batch, stems,
           bonds, bonds_batch):
    per_atom_out = np.asarray(per_atom_out, dtype=np.float32)
    per_mol_out = np.asarray(per_mol_out)
    slices_np = np.asarray(slices, dtype=np.int64)
    stems_batch = np.asarray(stems_batch, dtype=np.int64)
    stems = np.asarray(stems, dtype=np.int64)
    bonds = np.asarray(bonds, dtype=np.int64)
    bonds_batch = np.asarray(bonds_batch, dtype=np.int64)
    n_stems = stems_batch.shape[0]
    n_bonds = bonds_batch.shape[0]

    # ---- host: per-core stem binning ----
    stem_gidx = slices_np[stems_batch] + stems
    owner_s = stem_gidx // ROWS_PER_CORE

    per_core = []
    for c in range(NCORES):
        sel_s = np.flatnonzero(owner_s == c)
        loc_s = stem_gidx[sel_s] - c * ROWS_PER_CORE
        bkt_s = loc_s >> 15
        o = np.argsort(bkt_s, kind="stable")
        sel_s, loc_s = sel_s[o], loc_s[o]
        ns0 = int((loc_s < BUCKET).sum())
        per_core.append((sel_s, loc_s, ns0))

    NS0 = max(512, _round_up(max(pc[2] for pc in per_core), 512))
    NS1 = max(512, _round_up(max(len(pc[1]) - pc[2] for pc in per_core), 512))
    NST = NS0 + NS1

    # ---- host: weights/bias prep (shared across cores) ----
    w_s1 = np.asarray(w_s1, np.float32); w_b1 = np.asarray(w_b1, np.float32)
    w_s2 = np.asarray(w_s2, np.float32); w_b2 = np.asarray(w_b2, np.float32)
    b_s1 = np.asarray(b_s1, np.float32); b_b1 = np.asarray(b_b1, np.float32)
    b_s2 = np.asarray(b_s2, np.float32); b_b2 = np.asarray(b_b2, np.float32)

    def prep_w(w, d2):
        return np.ascontiguousarray(
            w.T.reshape(2, 128, d2).transpose(1, 0, 2).astype(BF16))

    w1s_a = prep_w(w_s1, DIM)
    w1b_a = prep_w(w_b1, DIM)
    w2s_a = prep_w(w_s2, S_DIM2)
    w2b_a = prep_w(0.5 * w_b2, 1)
    b1s_a = np.ascontiguousarray(b_s1.reshape(2, 128).T.astype(np.float32))
    b1b_a = np.ascontiguousarray(b_b1.reshape(2, 128).T.astype(np.float32))
    bs2_a = np.ascontiguousarray(b_s2.reshape(S_DIM2, 1).astype(np.float32))

    table_bf16 = per_atom_out.astype(BF16)

    # ---- host: per-core in_maps ----
    in_maps = []
    for c in range(NCORES):
        sel_s, loc_s, ns0 = per_core[c]
        ns1 = len(loc_s) - ns0

        sidx = np.zeros(NST, np.int16)
        sidx[:ns0] = loc_s[:ns0] & (BUCKET - 1)
        sidx[NS0 : NS0 + ns1] = loc_s[ns0:] & (BUCKET - 1)

        shard = table_bf16[c * ROWS_PER_CORE : (c + 1) * ROWS_PER_CORE]
        xt = np.ascontiguousarray(
            shard.T.reshape(2, 128, ROWS_PER_CORE).transpose(1, 0, 2))

        in_maps.append({
            "table": shard,
            "x_t": xt,
            "stem_idx": _wrap16(sidx),
            "w1s": w1s_a, "w1b": w1b_a, "w2s": w2s_a, "w2b": w2b_a,
            "b1s": b1s_a, "b1b": b1b_a, "bs2": bs2_a,
        })

    nc = _get_graph((NS0, NS1))
    global LAST_RUN
    LAST_RUN = (nc, in_maps)
    results = run_bass_kernel_spmd(nc, in_maps, core_ids=list(range(NCORES))).results

    # ---- host: assemble outputs ----
    stem_preds = np.empty((n_stems, S_DIM2), np.float32)
    P = np.empty(NCORES * ROWS_PER_CORE, np.float32)
    for c in range(NCORES):
        sel_s, loc_s, ns0 = per_core[c]
        ns1 = len(loc_s) - ns0
        so = results[c]["stem_out"]          # [105, NST]
        colmap = np.concatenate([np.arange(ns0), NS0 + np.arange(ns1)])
        stem_preds[sel_s] = so[:, colmap].T

        bo = results[c]["bond_out"]          # [G, 128, 256]
        P[c * ROWS_PER_CORE : (c + 1) * ROWS_PER_CORE] = (
            bo.transpose(0, 2, 1).reshape(-1))

    ga = slices_np[bonds_batch] + bonds[:, 0]
    gb = slices_np[bonds_batch] + bonds[:, 1]
    bonds_preds = (P[ga] + P[gb] + b_b2[0]).reshape(-1, 1).astype(np.float32)

    return (stem_preds, per_mol_out, bonds_preds)
